# revision 14
# baseline (speedup 1.0000x reference)
"""Trainium2 Bass kernel for nn_AnswerEmbedding (conv-seq2seq decoder block).

Strategy: pure data-parallel over batch (64 -> 8 per NeuronCore), no
collectives.  All activations stay on-chip in channel-major layout
[C, B*T]; weights are pre-transposed host-side into lhsT layouts.

Precision: the model is chaotic — the attention softmax sharpens over 8
layers and amplifies per-element noise ~600x into the output, so every
in-layer matmul runs in true fp32 (4 cycles/row on the PE).  Only the
final h2e+fc projections (whose errors do not amplify) run as float32r
(TF32-class, 1 cycle/row at N>=256).
"""

import os
import sys

import numpy as np
import ml_dtypes

for _p in ("/opt/trn_rl_repo",):
    if os.path.isdir(_p) and _p not in sys.path:
        sys.path.insert(0, _p)

import concourse.bass as bass
from concourse import bacc, masks, mybir, tile
from concourse.bass_utils import run_bass_kernel_spmd

B, SRC, EMB, HID, TRG, OUT, NL, K = 64, 100, 512, 1024, 100, 10000, 8, 3
CORES = 8
BL = B // CORES          # batches per core
BT = BL * TRG            # 800 local columns
TP = TRG + 2             # padded columns per batch
NB = 2                   # column chunks for N<=512 matmuls
NC_ = BT // NB           # 400 columns per chunk
BPC = BL // NB           # 4 batches per chunk
SCALE = float(np.float32(np.sqrt(np.float32(0.3))))
S2 = SCALE * SCALE

F32 = mybir.dt.float32
F32R = mybir.dt.float32r
BF = mybir.dt.bfloat16
BF16 = ml_dtypes.bfloat16
AF = mybir.ActivationFunctionType
OP = mybir.AluOpType
AX = mybir.AxisListType

ME = EMB // 128          # 4  E tiles
MH = HID // 128          # 8  HID tiles
NTAP = K * MH            # 24 contraction tiles per conv output tile (direct form)
NWI = 4                  # Winograd F(2,3) transforms
WJ = TRG // 2            # 50 output pairs per batch
WN = BPC * WJ            # 200 columns per Winograd matmul (per n-chunk)
NFC = 20                 # fc column chunks
WFC = OUT // NFC         # 500
MBT = (BT + 127) // 128  # 7 bt tiles for fc


def build(n_layers=NL):
    nc = bacc.Bacc(None, target_bir_lowering=False, debug=False)

    # ---- DRAM parameters (per-core shards / replicated weights) ----
    d_encT = nc.declare_dram_parameter("encT", [EMB, BT], F32, isOutput=False)
    d_encC = nc.declare_dram_parameter("encC", [BL, SRC, EMB], F32, isOutput=False)
    d_pos = nc.declare_dram_parameter("posT", [EMB, TRG], F32, isOutput=False)
    d_tgtW = nc.declare_dram_parameter("tgtWT", [SRC, TRG], F32, isOutput=False)
    d_tgtb = nc.declare_dram_parameter("tgtb", [TRG, 1], F32, isOutput=False)
    d_tokW = nc.declare_dram_parameter("tokWT", [TRG, EMB], F32, isOutput=False)
    d_tokb = nc.declare_dram_parameter("tokb", [128, ME], F32, isOutput=False)
    d_e2h = nc.declare_dram_parameter("e2hWT", [EMB, HID], F32, isOutput=False)
    d_e2hb = nc.declare_dram_parameter("e2hb", [128, MH], F32, isOutput=False)
    d_ah2e = nc.declare_dram_parameter("ah2eWT", [HID, EMB], F32, isOutput=False)
    d_ah2eb = nc.declare_dram_parameter("ah2eb", [128, ME], F32, isOutput=False)
    d_ae2h = nc.declare_dram_parameter("ae2hWT", [EMB, HID], F32, isOutput=False)
    d_ae2hb = nc.declare_dram_parameter("ae2hbs", [128, MH], F32, isOutput=False)
    d_h2e = nc.declare_dram_parameter("h2eWT", [HID, EMB], F32, isOutput=False)
    d_h2eb = nc.declare_dram_parameter("h2eb", [128, ME], F32, isOutput=False)
    d_fcW = nc.declare_dram_parameter("fcWT", [EMB, OUT], BF, isOutput=False)
    d_fcb = nc.declare_dram_parameter("fcb", [1, OUT], BF, isOutput=False)
    # [l, out-tile(16), p(128 contraction rows), (k, kk, q) = 24*128 cols]
    d_convW = nc.declare_dram_parameter(
        "convWT", [NL, 2 * MH, 128, NWI * MH * 128], F32, isOutput=False
    )
    d_cba = nc.declare_dram_parameter("cba", [NL, 128, MH], F32, isOutput=False)
    d_cbg = nc.declare_dram_parameter("cbg", [NL, 128, MH], F32, isOutput=False)

    d_out = nc.declare_dram_parameter("out", [BT, OUT], F32, isOutput=True)
    d_att = nc.declare_dram_parameter("att_out", [BL, TRG, SRC], F32, isOutput=True)

    with tile.TileContext(nc) as tc:
        with (
            tc.tile_pool(name="wp", bufs=1) as wp,
            tc.tile_pool(name="st", bufs=1) as st,
            tc.tile_pool(name="cw", bufs=3) as cw,
            tc.tile_pool(name="cb", bufs=2) as cb,
            tc.tile_pool(name="fw", bufs=5) as fw,
            tc.tile_pool(name="tp", bufs=2) as tp,
            tc.tile_pool(name="sp", bufs=2) as sp,
            tc.tile_pool(name="ps2", bufs=2, space=bass.MemorySpace.PSUM) as ps2,
            tc.tile_pool(name="ps1", bufs=1, space=bass.MemorySpace.PSUM) as ps1,
        ):
            CWW = MH * 128  # 1024 cols per conv weight chunk (one transform)

            # ---- static weights into SBUF ----
            def loadw(dram, parts, width, tagn):
                tiles = []
                for i in range(parts):
                    t = wp.tile([128, width], F32, tag=f"{tagn}{i}", name=f"{tagn}{i}")
                    nc.sync.dma_start(t[:], dram[i * 128:(i + 1) * 128, :])
                    tiles.append(t)
                return tiles

            ah2e_w = loadw(d_ah2e, MH, EMB, "ah2e")
            ae2h_w = loadw(d_ae2h, ME, HID, "ae2h")
            pos_w = loadw(d_pos, ME, TRG, "pos")
            encT = loadw(d_encT, ME, BT, "encT")

            tgt_w = wp.tile([SRC, TRG], F32, tag="tgtw", name="tgt_w")
            nc.sync.dma_start(tgt_w[:], d_tgtW[:])
            tok_w = wp.tile([TRG, EMB], F32, tag="tokw", name="tok_w")
            nc.sync.dma_start(tok_w[:], d_tokW[:])
            tgt_b = wp.tile([TRG, 1], F32, tag="tgtb", name="tgt_b")
            nc.sync.dma_start(tgt_b[:], d_tgtb[:])
            tok_b = wp.tile([128, ME], F32, tag="tokb", name="tok_b")
            nc.sync.dma_start(tok_b[:], d_tokb[:])
            e2h_b = wp.tile([128, MH], F32, tag="e2hb", name="e2h_b")
            nc.sync.dma_start(e2h_b[:], d_e2hb[:])
            ah2e_b = wp.tile([128, ME], F32, tag="ah2eb", name="ah2e_b")
            nc.sync.dma_start(ah2e_b[:], d_ah2eb[:])
            ae2h_bs = wp.tile([128, MH], F32, tag="ae2hbs", name="ae2h_bs")
            nc.sync.dma_start(ae2h_bs[:], d_ae2hb[:])
            h2e_b = wp.tile([128, ME], F32, tag="h2eb", name="h2e_b")
            nc.sync.dma_start(h2e_b[:], d_h2eb[:])

            ident = wp.tile([128, 128], F32, tag="ident", name="ident")
            masks.make_identity(nc, ident[:])
            ones_e = wp.tile([128, 1], F32, tag="ones_e", name="ones_e")
            nc.gpsimd.memset(ones_e[:], 1.0 / EMB)
            ones_1 = wp.tile([1, 128], BF, tag="ones_1", name="ones_1")
            nc.gpsimd.memset(ones_1[:], 1.0)

            # ---- persistent activation state ----
            # xf: fp32 residual stream in zero-padded layout [128, 8*(100+2)]
            xf = [st.tile([128, BL * TP], F32, tag=f"xf{i}", name=f"xf{i}") for i in range(MH)]
            glu = [st.tile([128, BT], F32, tag=f"glu{i}", name=f"glu{i}") for i in range(MH)]
            emb = [st.tile([128, BT], F32, tag=f"emb{i}", name=f"emb{i}") for i in range(ME)]
            comb = [st.tile([128, BT], F32, tag=f"comb{i}", name=f"comb{i}") for i in range(ME)]
            attcm = [st.tile([128, BT], F32, tag=f"attcm{i}", name=f"attcm{i}") for i in range(ME)]
            tok_sb = [st.tile([128, BL], F32, tag=f"tok{i}", name=f"tok{i}") for i in range(ME)]
            cef = [st.tile([128, BT], BF, tag=f"cef{i}", name=f"cef{i}") for i in range(ME)]

            for i in range(MH):
                nc.gpsimd.memset(xf[i][:], 0.0)

            def xf3(i):
                return xf[i][:].rearrange("p (b t) -> p b t", t=TP)

            def xslice(i, n, k=1, w=TRG):
                # data columns of chunk n with tap offset k (k=1 => aligned)
                return xf3(i)[:, n * BPC:(n + 1) * BPC, k:k + w]

            # ---- prologue: ec_mean -> hidden_target -> tok -> embedded ----
            psec = ps1.tile([SRC, SRC], F32, tag="en", name="psec")
            for b in range(BL):
                for ke in range(ME):
                    nc.tensor.matmul(
                        psec[:, b:b + 1],
                        encT[ke][:, b * TRG:(b + 1) * TRG],
                        ones_e[:],
                        start=(ke == 0), stop=(ke == ME - 1),
                    )
            ec_sb = sp.tile([SRC, BL], F32, tag="ec", name="ec_sb")
            nc.scalar.copy(ec_sb[:], psec[:, :BL])

            psht = ps1.tile([SRC, SRC], F32, tag="en", name="psht")
            nc.tensor.matmul(psht[:TRG, :BL], tgt_w[:], ec_sb[:], start=True, stop=True)
            ht_sb = sp.tile([TRG, BL], F32, tag="ht", name="ht_sb")
            nc.scalar.activation(ht_sb[:], psht[:TRG, :BL], AF.Identity, bias=tgt_b[:])

            for me in range(ME):
                pstok = ps2.tile([128, WFC], F32, tag="ce", name="pstok")
                nc.tensor.matmul(
                    pstok[:, :BL], tok_w[:, me * 128:(me + 1) * 128], ht_sb[:],
                    start=True, stop=True,
                )
                nc.scalar.activation(
                    tok_sb[me][:], pstok[:, :BL], AF.Identity,
                    bias=tok_b[:, me:me + 1],
                )

            for me in range(ME):
                for b in range(BL):
                    nc.scalar.activation(
                        emb[me][:, b * TRG:(b + 1) * TRG], pos_w[me][:],
                        AF.Identity, bias=tok_sb[me][:, b:b + 1],
                    )

            # conv_input = e2h(embedded); e2h weights stream through cw slots
            e2h_w = []
            for ke in range(ME):
                t = cw.tile([128, CWW], F32, tag=("wa" if ke < 2 else "wg"),
                            name=f"e2hw{ke}", bufs=3)
                nc.sync.dma_start(t[:], d_e2h[ke * 128:(ke + 1) * 128, :])
                e2h_w.append(t)
            for kk in range(MH):
                for n in range(NB):
                    pse = ps2.tile([128, WFC], F32, tag="ce", name="pse")
                    for ke in range(ME):
                        nc.tensor.matmul(
                            pse[:, :NC_],
                            e2h_w[ke][:, kk * 128:(kk + 1) * 128],
                            emb[ke][:, n * NC_:(n + 1) * NC_],
                            start=(ke == 0), stop=(ke == ME - 1),
                        )
                    nc.scalar.activation(
                        xslice(kk, n),
                        pse[:, :NC_].rearrange("p (b t) -> p b t", t=TRG),
                        AF.Identity, bias=e2h_b[:, kk:kk + 1],
                    )

            # ---- layers ----
            for l in range(n_layers):
                cba = cb.tile([128, MH], F32, tag="cba", name="cba")
                nc.sync.dma_start(cba[:], d_cba[l])
                cbg = cb.tile([128, MH], F32, tag="cbg", name="cbg")
                nc.sync.dma_start(cbg[:], d_cbg[l])

                # conv + GLU via Winograd F(2,3): y[2j]=m1+m2+m3,
                # y[2j+1]=m2-m3-m4, mi = Gi(W) . Bi(x) over channels.
                scr = comb + attcm  # scratch for transformed inputs
                for n in range(NB):
                    br = slice(n * BPC, (n + 1) * BPC)
                    for kk in range(MH):
                        xf4 = xf[kk][:].rearrange("p (b u v) -> p b u v", u=TP // 2, v=2)
                        d0 = xf4[:, br, 0:WJ, 0]
                        d1 = xf4[:, br, 0:WJ, 1]
                        d2 = xf4[:, br, 1:WJ + 1, 0]
                        d3 = xf4[:, br, 1:WJ + 1, 1]
                        for i, (pa, pb, op) in enumerate(
                            [(d0, d2, OP.subtract), (d1, d2, OP.add),
                             (d2, d1, OP.subtract), (d1, d3, OP.subtract)]
                        ):
                            o3 = scr[kk][:, i * WN:(i + 1) * WN].rearrange(
                                "p (b j) -> p b j", j=WJ)
                            nc.vector.scalar_tensor_tensor(
                                o3, pa, 0.0, pb, OP.bypass, op)
                    for m in range(MH):
                        gl4 = glu[m][:].rearrange("p (b u v) -> p b u v", u=WJ, v=2)
                        # a-half -> SBUF temps, g-half stays in PSUM
                        psA = []
                        for i in range(NWI):
                            wa = cw.tile([128, CWW], F32, tag="wa", name="wa", bufs=3)
                            nc.sync.dma_start(
                                wa[:], d_convW[l, m, :, i * CWW:(i + 1) * CWW])
                            ps = ps2.tile([128, WN], F32, tag=f"m{i}",
                                          name=f"psA{i}", bufs=1)
                            for kk in range(MH):
                                nc.tensor.matmul(
                                    ps[:], wa[:, kk * 128:(kk + 1) * 128],
                                    scr[kk][:, i * WN:(i + 1) * WN],
                                    start=(kk == 0), stop=(kk == MH - 1),
                                )
                            psA.append(ps)
                        tva = tp.tile([128, WN], F32, tag="tva", name="tva")
                        nc.vector.scalar_tensor_tensor(
                            tva[:], psA[0][:], 0.0, psA[1][:], OP.bypass, OP.add)
                        nc.vector.scalar_tensor_tensor(
                            tva[:], tva[:], 0.0, psA[2][:], OP.bypass, OP.add)
                        tvb = tp.tile([128, WN], F32, tag="tvb", name="tvb")
                        nc.vector.scalar_tensor_tensor(
                            tvb[:], psA[1][:], 0.0, psA[2][:], OP.bypass, OP.subtract)
                        nc.vector.scalar_tensor_tensor(
                            tvb[:], tvb[:], 0.0, psA[3][:], OP.bypass, OP.subtract)
                        psG = []
                        for i in range(NWI):
                            wg = cw.tile([128, CWW], F32, tag="wg", name="wg", bufs=3)
                            nc.sync.dma_start(
                                wg[:], d_convW[l, MH + m, :, i * CWW:(i + 1) * CWW])
                            ps = ps2.tile([128, WN], F32, tag=f"m{i}",
                                          name=f"psG{i}", bufs=1)
                            for kk in range(MH):
                                nc.tensor.matmul(
                                    ps[:], wg[:, kk * 128:(kk + 1) * 128],
                                    scr[kk][:, i * WN:(i + 1) * WN],
                                    start=(kk == 0), stop=(kk == MH - 1),
                                )
                            psG.append(ps)
                        nc.vector.scalar_tensor_tensor(
                            psG[0][:], psG[0][:], 0.0, psG[1][:], OP.bypass, OP.add)
                        nc.vector.scalar_tensor_tensor(
                            psG[1][:], psG[1][:], 0.0, psG[2][:], OP.bypass, OP.subtract)
                        nc.vector.scalar_tensor_tensor(
                            psG[0][:], psG[0][:], 0.0, psG[2][:], OP.bypass, OP.add)
                        nc.vector.scalar_tensor_tensor(
                            psG[1][:], psG[1][:], 0.0, psG[3][:], OP.bypass, OP.subtract)
                        sigv = tp.tile([128, WN], F32, tag="sigv", name="sigv")
                        nc.scalar.activation(
                            sigv[:], psG[0][:], AF.Sigmoid, bias=cbg[:, m:m + 1])
                        nc.vector.scalar_tensor_tensor(
                            gl4[:, br, :, 0], tva[:], cba[:, m:m + 1], sigv[:],
                            OP.add, OP.mult)
                        sigo = tp.tile([128, WN], F32, tag="sigo", name="sigo")
                        nc.scalar.activation(
                            sigo[:], psG[1][:], AF.Sigmoid, bias=cbg[:, m:m + 1])
                        nc.vector.scalar_tensor_tensor(
                            gl4[:, br, :, 1], tvb[:], cba[:, m:m + 1], sigo[:],
                            OP.add, OP.mult)

                # conved_emb + embedded (combined, unscaled)
                for me in range(ME):
                    for n in range(NB):
                        cs = slice(n * NC_, (n + 1) * NC_)
                        pc = ps2.tile([128, WFC], F32, tag="ce", name="pc")
                        for kk in range(MH):
                            nc.tensor.matmul(
                                pc[:, :NC_],
                                ah2e_w[kk][:, me * 128:(me + 1) * 128],
                                glu[kk][:, cs],
                                start=(kk == 0), stop=(kk == MH - 1),
                            )
                        nc.vector.scalar_tensor_tensor(
                            comb[me][:, cs], pc[:, :NC_], ah2e_b[:, me:me + 1],
                            emb[me][:, cs], OP.add, OP.add,
                        )

                # attention per batch
                for b in range(BL):
                    bs = slice(b * TRG, (b + 1) * TRG)
                    encC = wp.tile([SRC, EMB], F32, tag="encC", name="encC", bufs=3)
                    nc.sync.dma_start(encC[:], d_encC[b])
                    pen = ps1.tile([TRG, SRC], F32, tag="en", name="pen")
                    for ke in range(ME):
                        nc.tensor.matmul(
                            pen[:], comb[ke][:, bs], encT[ke][:, bs],
                            start=(ke == 0), stop=(ke == ME - 1),
                        )
                    mx = sp.tile([TRG, 1], F32, tag="mx", name="mx")
                    nc.vector.reduce_max(mx[:], pen[:], AX.X)
                    ngb = sp.tile([TRG, 1], F32, tag="ngb", name="ngb")
                    nc.scalar.mul(ngb[:], mx[:], -SCALE)
                    ex = sp.tile([TRG, SRC], F32, tag="ex", name="ex")
                    sm = sp.tile([TRG, 1], F32, tag="sm", name="sm")
                    nc.scalar.activation(
                        ex[:], pen[:], AF.Exp, bias=ngb[:], scale=SCALE,
                        accum_out=sm[:],
                    )
                    rc = sp.tile([TRG, 1], F32, tag="rc", name="rc")
                    nc.vector.reciprocal(rc[:], sm[:])
                    att = sp.tile([TRG, SRC], F32, tag="att", name="att")
                    nc.vector.tensor_scalar_mul(att[:], ex[:], rc[:])
                    if l == n_layers - 1:
                        nc.sync.dma_start(d_att[b], att[:])
                    ptT = ps1.tile([SRC, TRG], F32, tag="tT", name="ptT")
                    nc.tensor.transpose(ptT[:], att[:], ident[:TRG, :TRG])
                    atT = sp.tile([SRC, TRG], F32, tag="atT", name="atT")
                    nc.scalar.copy(atT[:], ptT[:])
                    for me in range(ME):
                        pat = ps2.tile([128, WFC], F32, tag="ce", name="pat")
                        nc.tensor.matmul(
                            pat[:, :TRG], encC[:, me * 128:(me + 1) * 128],
                            atT[:], start=True, stop=True,
                        )
                        nc.scalar.copy(attcm[me][:, bs], pat[:, :TRG])

                # ae2h + residual epilogue:
                # x' = x*S + glu*S^2 + (ae2h(attended) + ae2h_b)*S^2
                for m in range(MH):
                    for n in range(NB):
                        cs = slice(n * NC_, (n + 1) * NC_)
                        pah = ps2.tile([128, WFC], F32, tag="ce", name="pah")
                        for ke in range(ME):
                            nc.tensor.matmul(
                                pah[:, :NC_],
                                ae2h_w[ke][:, m * 128:(m + 1) * 128],
                                attcm[ke][:, cs],
                                start=(ke == 0), stop=(ke == ME - 1),
                            )
                        s1 = tp.tile([128, NC_], F32, tag="s1", name="s1")
                        nc.scalar.activation(
                            s1[:], pah[:, :NC_], AF.Identity,
                            bias=ae2h_bs[:, m:m + 1], scale=S2,
                        )
                        nc.vector.scalar_tensor_tensor(
                            s1[:], glu[m][:, cs], S2, s1[:], OP.mult, OP.add
                        )
                        nc.vector.scalar_tensor_tensor(
                            xslice(m, n), xslice(m, n), SCALE,
                            s1[:].rearrange("p (b t) -> p b t", t=TRG),
                            OP.mult, OP.add,
                        )

            # ---- h2e (float32r), contraction split so only 4 weight tiles
            # are live at a time (2 per streaming tag) ----
            for half in range(2):
                h2e_w = []
                for j in range(4):
                    kk = half * 4 + j
                    t = cw.tile([128, CWW], F32, tag=("wa" if j < 2 else "wg"),
                                name=f"h2ew{kk}", bufs=3)
                    nc.sync.dma_start(t[:, :EMB], d_h2e[kk * 128:(kk + 1) * 128, :])
                    h2e_w.append(t)
                for me in range(ME):
                    for n in range(NB):
                        cs = slice(n * NC_, (n + 1) * NC_)
                        ph = ps2.tile([128, WFC], F32, tag="ce", name="ph")
                        for j in range(4):
                            kk = half * 4 + j
                            nc.tensor.matmul(
                                ph[:, :NC_],
                                h2e_w[j][:, me * 128:(me + 1) * 128],
                                xslice(kk, n),
                                start=(j == 0), stop=(j == 3),
                            )
                        if half == 0:
                            nc.scalar.activation(
                                cef[me][:, cs], ph[:, :NC_], AF.Identity,
                                bias=h2e_b[:, me:me + 1],
                            )
                        else:
                            nc.vector.scalar_tensor_tensor(
                                cef[me][:, cs], ph[:, :NC_], 0.0,
                                cef[me][:, cs], OP.bypass, OP.add,
                            )

            # ---- fc_out (float32r) ----
            for n in range(NFC):
                ns = slice(n * WFC, (n + 1) * WFC)
                fcb_t = fw.tile([1, WFC], BF, tag="fcb", name="fcb_t", bufs=2)
                nc.sync.dma_start(fcb_t[:], d_fcb[:, ns])
                fws = []
                for ke in range(ME):
                    t = fw.tile([128, WFC], BF, tag="fcw", name=f"fcw{ke}")
                    nc.sync.dma_start(t[:], d_fcW[ke * 128:(ke + 1) * 128, ns])
                    fws.append(t)
                for mb in range(MBT):
                    mw = min(128, BT - mb * 128)
                    pf = ps2.tile([128, WFC], F32, tag="ce", name="pf")
                    for ke in range(ME):
                        nc.tensor.matmul(
                            pf[:mw, :],
                            cef[ke][:, mb * 128:mb * 128 + mw],
                            fws[ke][:],
                            start=(ke == 0), stop=False,
                        )
                    nc.tensor.matmul(
                        pf[:mw, :], ones_1[:, :mw],
                        fcb_t[:],
                        start=False, stop=True,
                    )
                    fo = fw.tile([128, WFC], F32, tag="fo", name="fo", bufs=2)
                    nc.vector.tensor_copy(fo[:mw, :], pf[:mw, :])
                    nc.sync.dma_start(d_out[mb * 128:mb * 128 + mw, ns], fo[:mw, :])

    nc.compile()
    return nc


_CACHED = {}


def _get_nc(n_layers=NL):
    if n_layers not in _CACHED:
        _CACHED[n_layers] = build(n_layers)
    return _CACHED[n_layers]


def _prep_weights(i):
    """Host-side weight preprocessing shared by all cores."""
    f32 = np.float32
    asf = lambda x: np.asarray(x, f32)
    w = {}
    w["posT"] = np.ascontiguousarray(asf(i["pos_emb"]).T)
    w["tgtWT"] = np.ascontiguousarray(asf(i["tgt_W"]).T)
    w["tgtb"] = asf(i["tgt_b"]).reshape(TRG, 1)
    w["tokWT"] = np.ascontiguousarray(asf(i["tok_W"]).T)
    w["tokb"] = np.ascontiguousarray(asf(i["tok_b"]).reshape(ME, 128).T)
    w["e2hWT"] = np.ascontiguousarray(asf(i["e2h_W"]).T)
    w["e2hb"] = np.ascontiguousarray(asf(i["e2h_b"]).reshape(MH, 128).T)
    w["ah2eWT"] = np.ascontiguousarray(asf(i["ah2e_W"]).T)
    w["ah2eb"] = np.ascontiguousarray(asf(i["ah2e_b"]).reshape(ME, 128).T)
    w["ae2hWT"] = np.ascontiguousarray(asf(i["ae2h_W"]).T)
    w["ae2hbs"] = np.ascontiguousarray(
        (asf(i["ae2h_b"]) * np.float32(S2)).reshape(MH, 128).T
    )
    w["h2eWT"] = np.ascontiguousarray(asf(i["h2e_W"]).T)
    w["h2eb"] = np.ascontiguousarray(asf(i["h2e_b"]).reshape(ME, 128).T)
    w["fcWT"] = np.ascontiguousarray(asf(i["fc_W"]).T).astype(BF16)
    w["fcb"] = asf(i["fc_b"]).reshape(1, OUT).astype(BF16)
    cW = asf(i["conv_W"])  # [NL, 2H, H, K]
    g0, g1, g2 = cW[..., 0], cW[..., 1], cW[..., 2]
    cw4 = np.stack(
        [g0, (g0 + g1 + g2) * np.float32(0.5),
         (g0 - g1 + g2) * np.float32(0.5), g2], axis=-1)  # [NL, 2H, H, 4]
    # [l, mt, q, kk, p, i] -> [l, mt, p, (i, kk, q)]
    w["convWT"] = np.ascontiguousarray(
        cw4.reshape(NL, 2 * MH, 128, MH, 128, NWI).transpose(0, 1, 4, 5, 3, 2)
    ).reshape(NL, 2 * MH, 128, NWI * MH * 128)
    cb_ = asf(i["conv_b"])  # [NL, 2H]
    w["cba"] = np.ascontiguousarray(cb_[:, :HID].reshape(NL, MH, 128).transpose(0, 2, 1))
    w["cbg"] = np.ascontiguousarray(cb_[:, HID:].reshape(NL, MH, 128).transpose(0, 2, 1))
    return w


LAST_EXEC_NS = None


def _maybe_enable_trace():
    """Register the NTFF profile hook (missing antenv.axon_hooks shim)."""
    try:
        import antenv.axon_hooks  # noqa: F401
        return True
    except ImportError:
        pass
    try:
        import types
        import antenv
        from trn_agent_boot.trn_boot import _ntff_profile_via_ctypes

        hook = _ntff_profile_via_ctypes("/opt/axon/libaxon_pjrt.so")
        mod = types.ModuleType("antenv.axon_hooks")
        _state = {"hook": hook}
        mod.set_axon_ntff_profile_hook = lambda h: _state.__setitem__("hook", h)
        mod.get_axon_ntff_profile_hook = lambda: _state["hook"]
        sys.modules["antenv.axon_hooks"] = mod
        antenv.axon_hooks = mod
        return hook is not None
    except Exception:
        return False


def kernel(**inputs):
    global LAST_EXEC_NS
    n_layers = int(os.environ.get("KERNEL_NL", NL))
    trace = os.environ.get("KERNEL_TRACE", "0") == "1"
    if trace:
        trace = _maybe_enable_trace()

    nc = _get_nc(n_layers)
    w = _prep_weights(inputs)

    enc_conved = np.asarray(inputs["encoder_conved"], np.float32)
    enc_combined = np.asarray(inputs["encoder_combined"], np.float32)

    in_maps = []
    for c in range(CORES):
        sh = slice(c * BL, (c + 1) * BL)
        m = dict(w)
        m["encT"] = np.ascontiguousarray(
            enc_conved[sh].transpose(2, 0, 1).reshape(EMB, BT)
        )
        m["encC"] = np.ascontiguousarray(enc_combined[sh])
        in_maps.append(m)

    res = run_bass_kernel_spmd(
        nc, in_maps, list(range(CORES)), trace=trace,
        trace_cores=[0] if trace else None,
    )
    LAST_EXEC_NS = res.exec_time_ns

    out = np.empty((B, TRG, OUT), np.float32)
    att = np.empty((B, TRG, SRC), np.float32)
    for c in range(CORES):
        out[c * BL:(c + 1) * BL] = res.results[c]["out"].reshape(BL, TRG, OUT)
        att[c * BL:(c + 1) * BL] = res.results[c]["att_out"]
    return out, att


# revision 15
# speedup vs baseline: 1.3253x; 1.3253x over previous
"""Trainium2 Bass kernel for nn_AnswerEmbedding (conv-seq2seq decoder block).

Strategy: pure data-parallel over batch (64 -> 8 per NeuronCore), no
collectives.  All activations stay on-chip in channel-major layout
[C, B*T]; weights are pre-transposed host-side into lhsT layouts.

Precision: the model is chaotic — the attention softmax sharpens over 8
layers and amplifies per-element noise ~600x into the output, so every
in-layer matmul runs in true fp32 (4 cycles/row on the PE).  Only the
final h2e+fc projections (whose errors do not amplify) run as float32r
(TF32-class, 1 cycle/row at N>=256).
"""

import os
import sys

import numpy as np
import ml_dtypes

for _p in ("/opt/trn_rl_repo",):
    if os.path.isdir(_p) and _p not in sys.path:
        sys.path.insert(0, _p)

import concourse.bass as bass
from concourse import bacc, masks, mybir, tile
from concourse.bass_utils import run_bass_kernel_spmd

B, SRC, EMB, HID, TRG, OUT, NL, K = 64, 100, 512, 1024, 100, 10000, 8, 3
CORES = 8
BL = B // CORES          # batches per core
BT = BL * TRG            # 800 local columns
TP = TRG + 2             # padded columns per batch
NB = 2                   # column chunks for N<=512 matmuls
NC_ = BT // NB           # 400 columns per chunk
BPC = BL // NB           # 4 batches per chunk
SCALE = float(np.float32(np.sqrt(np.float32(0.3))))
S2 = SCALE * SCALE

F32 = mybir.dt.float32
F32R = mybir.dt.float32r
BF = mybir.dt.bfloat16
BF16 = ml_dtypes.bfloat16
AF = mybir.ActivationFunctionType
OP = mybir.AluOpType
AX = mybir.AxisListType

ME = EMB // 128          # 4  E tiles
MH = HID // 128          # 8  HID tiles
NTAP = K * MH            # 24 contraction tiles per conv output tile (direct form)
NWI = 4                  # Winograd F(2,3) transforms
WJ = TRG // 2            # 50 output pairs per batch
WN = BPC * WJ            # 200 columns per Winograd matmul (per n-chunk)
NFC = 20                 # fc column chunks
WFC = OUT // NFC         # 500
MBT = (BT + 127) // 128  # 7 bt tiles for fc


def build(n_layers=NL):
    nc = bacc.Bacc(None, target_bir_lowering=False, debug=False)

    # ---- DRAM parameters (per-core shards / replicated weights) ----
    d_encT = nc.declare_dram_parameter("encT", [EMB, BT], F32, isOutput=False)
    d_encC = nc.declare_dram_parameter("encC", [BL, SRC, EMB], F32, isOutput=False)
    d_pos = nc.declare_dram_parameter("posT", [EMB, TRG], F32, isOutput=False)
    d_tgtW = nc.declare_dram_parameter("tgtWT", [SRC, TRG], F32, isOutput=False)
    d_tgtb = nc.declare_dram_parameter("tgtb", [TRG, 1], F32, isOutput=False)
    d_tokW = nc.declare_dram_parameter("tokWT", [TRG, EMB], F32, isOutput=False)
    d_tokb = nc.declare_dram_parameter("tokb", [128, ME], F32, isOutput=False)
    d_e2h = nc.declare_dram_parameter("e2hWT", [EMB, HID], F32, isOutput=False)
    d_e2hb = nc.declare_dram_parameter("e2hb", [128, MH], F32, isOutput=False)
    d_ah2e = nc.declare_dram_parameter("ah2eWT", [HID, EMB], F32, isOutput=False)
    d_ah2eb = nc.declare_dram_parameter("ah2eb", [128, ME], F32, isOutput=False)
    d_ae2h = nc.declare_dram_parameter("ae2hWT", [EMB, HID], F32, isOutput=False)
    d_ae2hb = nc.declare_dram_parameter("ae2hbs", [128, MH], F32, isOutput=False)
    d_h2e = nc.declare_dram_parameter("h2eWT", [HID, EMB], F32, isOutput=False)
    d_h2eb = nc.declare_dram_parameter("h2eb", [128, ME], F32, isOutput=False)
    d_fcW = nc.declare_dram_parameter("fcWT", [EMB, OUT], BF, isOutput=False)
    d_fcb = nc.declare_dram_parameter("fcb", [1, OUT], BF, isOutput=False)
    # [l, out-tile(16), p(128 contraction rows), (k, kk, q) = 24*128 cols]
    d_convW = nc.declare_dram_parameter(
        "convWT", [NL, 2 * MH, 128, NWI * MH * 128], F32, isOutput=False
    )
    d_cba = nc.declare_dram_parameter("cba", [NL, 128, MH], F32, isOutput=False)
    d_cbg = nc.declare_dram_parameter("cbg", [NL, 128, MH], F32, isOutput=False)

    d_out = nc.declare_dram_parameter("out", [BT, OUT], F32, isOutput=True)
    d_att = nc.declare_dram_parameter("att_out", [BL, TRG, SRC], F32, isOutput=True)

    with tile.TileContext(nc) as tc:
        with (
            tc.tile_pool(name="wp", bufs=1) as wp,
            tc.tile_pool(name="st", bufs=1) as st,
            tc.tile_pool(name="cw", bufs=3) as cw,
            tc.tile_pool(name="cb", bufs=2) as cb,
            tc.tile_pool(name="fw", bufs=5) as fw,
            tc.tile_pool(name="tp", bufs=2) as tp,
            tc.tile_pool(name="sp", bufs=2) as sp,
            tc.tile_pool(name="ps2", bufs=2, space=bass.MemorySpace.PSUM) as ps2,
            tc.tile_pool(name="ps1", bufs=1, space=bass.MemorySpace.PSUM) as ps1,
        ):
            CWW = MH * 128  # 1024 cols per conv weight chunk (one transform)

            # ---- static weights into SBUF ----
            def loadw(dram, parts, width, tagn):
                tiles = []
                for i in range(parts):
                    t = wp.tile([128, width], F32, tag=f"{tagn}{i}", name=f"{tagn}{i}")
                    nc.sync.dma_start(t[:], dram[i * 128:(i + 1) * 128, :])
                    tiles.append(t)
                return tiles

            ah2e_w = loadw(d_ah2e, MH, EMB, "ah2e")
            ae2h_w = loadw(d_ae2h, ME, HID, "ae2h")
            pos_w = loadw(d_pos, ME, TRG, "pos")
            encT = loadw(d_encT, ME, BT, "encT")

            tgt_w = wp.tile([SRC, TRG], F32, tag="tgtw", name="tgt_w")
            nc.sync.dma_start(tgt_w[:], d_tgtW[:])
            tok_w = wp.tile([TRG, EMB], F32, tag="tokw", name="tok_w")
            nc.sync.dma_start(tok_w[:], d_tokW[:])
            tgt_b = wp.tile([TRG, 1], F32, tag="tgtb", name="tgt_b")
            nc.sync.dma_start(tgt_b[:], d_tgtb[:])
            tok_b = wp.tile([128, ME], F32, tag="tokb", name="tok_b")
            nc.sync.dma_start(tok_b[:], d_tokb[:])
            e2h_b = wp.tile([128, MH], F32, tag="e2hb", name="e2h_b")
            nc.sync.dma_start(e2h_b[:], d_e2hb[:])
            ah2e_b = wp.tile([128, ME], F32, tag="ah2eb", name="ah2e_b")
            nc.sync.dma_start(ah2e_b[:], d_ah2eb[:])
            ae2h_bs = wp.tile([128, MH], F32, tag="ae2hbs", name="ae2h_bs")
            nc.sync.dma_start(ae2h_bs[:], d_ae2hb[:])
            h2e_b = wp.tile([128, ME], F32, tag="h2eb", name="h2e_b")
            nc.sync.dma_start(h2e_b[:], d_h2eb[:])

            ident = wp.tile([128, 128], F32, tag="ident", name="ident")
            masks.make_identity(nc, ident[:])
            ones_e = wp.tile([128, 1], F32, tag="ones_e", name="ones_e")
            nc.gpsimd.memset(ones_e[:], 1.0 / EMB)
            ones_1 = wp.tile([1, 128], BF, tag="ones_1", name="ones_1")
            nc.gpsimd.memset(ones_1[:], 1.0)

            # ---- persistent activation state ----
            # xf: fp32 residual stream in zero-padded layout [128, 8*(100+2)]
            xf = [st.tile([128, BL * TP], F32, tag=f"xf{i}", name=f"xf{i}") for i in range(MH)]
            glu = [st.tile([128, BT], F32, tag=f"glu{i}", name=f"glu{i}") for i in range(MH)]
            emb = [st.tile([128, BT], F32, tag=f"emb{i}", name=f"emb{i}") for i in range(ME)]
            comb = [st.tile([128, BT], F32, tag=f"comb{i}", name=f"comb{i}") for i in range(ME)]
            attcm = [st.tile([128, BT], F32, tag=f"attcm{i}", name=f"attcm{i}") for i in range(ME)]
            tok_sb = [st.tile([128, BL], F32, tag=f"tok{i}", name=f"tok{i}") for i in range(ME)]
            cef = [st.tile([128, BT], BF, tag=f"cef{i}", name=f"cef{i}") for i in range(ME)]

            for i in range(MH):
                nc.gpsimd.memset(xf[i][:], 0.0)

            def xf3(i):
                return xf[i][:].rearrange("p (b t) -> p b t", t=TP)

            def xslice(i, n, k=1, w=TRG):
                # data columns of chunk n with tap offset k (k=1 => aligned)
                return xf3(i)[:, n * BPC:(n + 1) * BPC, k:k + w]

            # ---- prologue: ec_mean -> hidden_target -> tok -> embedded ----
            psec = ps1.tile([SRC, SRC], F32, tag="en", name="psec")
            for b in range(BL):
                for ke in range(ME):
                    nc.tensor.matmul(
                        psec[:, b:b + 1],
                        encT[ke][:, b * TRG:(b + 1) * TRG],
                        ones_e[:],
                        start=(ke == 0), stop=(ke == ME - 1),
                    )
            ec_sb = sp.tile([SRC, BL], F32, tag="ec", name="ec_sb")
            nc.scalar.copy(ec_sb[:], psec[:, :BL])

            psht = ps1.tile([SRC, SRC], F32, tag="en", name="psht")
            nc.tensor.matmul(psht[:TRG, :BL], tgt_w[:], ec_sb[:], start=True, stop=True)
            ht_sb = sp.tile([TRG, BL], F32, tag="ht", name="ht_sb")
            nc.scalar.activation(ht_sb[:], psht[:TRG, :BL], AF.Identity, bias=tgt_b[:])

            for me in range(ME):
                pstok = ps2.tile([128, WFC], F32, tag="ce", name="pstok")
                nc.tensor.matmul(
                    pstok[:, :BL], tok_w[:, me * 128:(me + 1) * 128], ht_sb[:],
                    start=True, stop=True,
                )
                nc.scalar.activation(
                    tok_sb[me][:], pstok[:, :BL], AF.Identity,
                    bias=tok_b[:, me:me + 1],
                )

            for me in range(ME):
                for b in range(BL):
                    nc.scalar.activation(
                        emb[me][:, b * TRG:(b + 1) * TRG], pos_w[me][:],
                        AF.Identity, bias=tok_sb[me][:, b:b + 1],
                    )

            # conv_input = e2h(embedded); e2h weights stream through cw slots
            e2h_w = []
            for ke in range(ME):
                t = cw.tile([128, CWW], F32, tag=("wa" if ke < 2 else "wg"),
                            name=f"e2hw{ke}", bufs=3)
                nc.sync.dma_start(t[:], d_e2h[ke * 128:(ke + 1) * 128, :])
                e2h_w.append(t)
            for kk in range(MH):
                for n in range(NB):
                    pse = ps2.tile([128, WFC], F32, tag="ce", name="pse")
                    for ke in range(ME):
                        nc.tensor.matmul(
                            pse[:, :NC_],
                            e2h_w[ke][:, kk * 128:(kk + 1) * 128],
                            emb[ke][:, n * NC_:(n + 1) * NC_],
                            start=(ke == 0), stop=(ke == ME - 1),
                        )
                    nc.scalar.activation(
                        xslice(kk, n),
                        pse[:, :NC_].rearrange("p (b t) -> p b t", t=TRG),
                        AF.Identity, bias=e2h_b[:, kk:kk + 1],
                    )

            # ---- layers ----
            for l in range(n_layers):
                cba = cb.tile([128, MH], F32, tag="cba", name="cba")
                nc.sync.dma_start(cba[:], d_cba[l])
                cbg = cb.tile([128, MH], F32, tag="cbg", name="cbg")
                nc.sync.dma_start(cbg[:], d_cbg[l])

                # conv + GLU via Winograd F(2,3): y[2j]=m1+m2+m3,
                # y[2j+1]=m2-m3-m4, mi = Gi(W) . Bi(x) over channels.
                scr = comb + attcm  # scratch for transformed inputs
                for n in range(NB):
                    br = slice(n * BPC, (n + 1) * BPC)
                    for kk in range(MH):
                        xf4 = xf[kk][:].rearrange("p (b u v) -> p b u v", u=TP // 2, v=2)
                        d0 = xf4[:, br, 0:WJ, 0]
                        d1 = xf4[:, br, 0:WJ, 1]
                        d2 = xf4[:, br, 1:WJ + 1, 0]
                        d3 = xf4[:, br, 1:WJ + 1, 1]
                        for i, (pa, pb, op) in enumerate(
                            [(d0, d2, OP.subtract), (d1, d2, OP.add),
                             (d2, d1, OP.subtract), (d1, d3, OP.subtract)]
                        ):
                            o3 = scr[kk][:, i * WN:(i + 1) * WN].rearrange(
                                "p (b j) -> p b j", j=WJ)
                            nc.vector.scalar_tensor_tensor(
                                o3, pa, 0.0, pb, OP.bypass, op)
                    for m in range(MH):
                        gl4 = glu[m][:].rearrange("p (b u v) -> p b u v", u=WJ, v=2)
                        # a-half -> SBUF temps, g-half stays in PSUM
                        psA = []
                        for i in range(NWI):
                            wa = cw.tile([128, CWW], F32, tag="wa", name="wa", bufs=3)
                            nc.sync.dma_start(
                                wa[:], d_convW[l, m, :, i * CWW:(i + 1) * CWW])
                            ps = ps2.tile([128, WN], F32, tag=f"m{i}",
                                          name=f"psA{i}", bufs=1)
                            for kk in range(MH):
                                nc.tensor.matmul(
                                    ps[:], wa[:, kk * 128:(kk + 1) * 128],
                                    scr[kk][:, i * WN:(i + 1) * WN],
                                    start=(kk == 0), stop=(kk == MH - 1),
                                )
                            psA.append(ps)
                        c1 = tp.tile([128, WN], F32, tag="cc", name="c1")
                        nc.vector.tensor_copy(c1[:], psA[1][:])
                        tva = tp.tile([128, WN], F32, tag="tva", name="tva")
                        nc.vector.scalar_tensor_tensor(
                            tva[:], psA[0][:], 0.0, c1[:], OP.bypass, OP.add)
                        nc.vector.scalar_tensor_tensor(
                            tva[:], psA[2][:], 0.0, tva[:], OP.bypass, OP.add)
                        tvb = tp.tile([128, WN], F32, tag="tvb", name="tvb")
                        nc.vector.scalar_tensor_tensor(
                            tvb[:], psA[2][:], -1.0, c1[:], OP.mult, OP.add)
                        nc.vector.scalar_tensor_tensor(
                            tvb[:], psA[3][:], -1.0, tvb[:], OP.mult, OP.add)
                        psG = []
                        for i in range(NWI):
                            wg = cw.tile([128, CWW], F32, tag="wg", name="wg", bufs=3)
                            nc.sync.dma_start(
                                wg[:], d_convW[l, MH + m, :, i * CWW:(i + 1) * CWW])
                            ps = ps2.tile([128, WN], F32, tag=f"m{i}",
                                          name=f"psG{i}", bufs=1)
                            for kk in range(MH):
                                nc.tensor.matmul(
                                    ps[:], wg[:, kk * 128:(kk + 1) * 128],
                                    scr[kk][:, i * WN:(i + 1) * WN],
                                    start=(kk == 0), stop=(kk == MH - 1),
                                )
                            psG.append(ps)
                        c2 = tp.tile([128, WN], F32, tag="cc", name="c2")
                        nc.vector.tensor_copy(c2[:], psG[1][:])
                        tvc = tp.tile([128, WN], F32, tag="tvc", name="tvc")
                        nc.vector.scalar_tensor_tensor(
                            tvc[:], psG[0][:], 0.0, c2[:], OP.bypass, OP.add)
                        nc.vector.scalar_tensor_tensor(
                            tvc[:], psG[2][:], 0.0, tvc[:], OP.bypass, OP.add)
                        tvd = tp.tile([128, WN], F32, tag="tvd", name="tvd")
                        nc.vector.scalar_tensor_tensor(
                            tvd[:], psG[2][:], -1.0, c2[:], OP.mult, OP.add)
                        nc.vector.scalar_tensor_tensor(
                            tvd[:], psG[3][:], -1.0, tvd[:], OP.mult, OP.add)
                        nc.scalar.activation(
                            tvc[:], tvc[:], AF.Sigmoid, bias=cbg[:, m:m + 1])
                        nc.vector.scalar_tensor_tensor(
                            gl4[:, br, :, 0], tva[:], cba[:, m:m + 1], tvc[:],
                            OP.add, OP.mult)
                        nc.scalar.activation(
                            tvd[:], tvd[:], AF.Sigmoid, bias=cbg[:, m:m + 1])
                        nc.vector.scalar_tensor_tensor(
                            gl4[:, br, :, 1], tvb[:], cba[:, m:m + 1], tvd[:],
                            OP.add, OP.mult)

                # conved_emb + embedded (combined, unscaled)
                for me in range(ME):
                    for n in range(NB):
                        cs = slice(n * NC_, (n + 1) * NC_)
                        pc = ps2.tile([128, WFC], F32, tag="ce", name="pc")
                        for kk in range(MH):
                            nc.tensor.matmul(
                                pc[:, :NC_],
                                ah2e_w[kk][:, me * 128:(me + 1) * 128],
                                glu[kk][:, cs],
                                start=(kk == 0), stop=(kk == MH - 1),
                            )
                        nc.vector.scalar_tensor_tensor(
                            comb[me][:, cs], pc[:, :NC_], ah2e_b[:, me:me + 1],
                            emb[me][:, cs], OP.add, OP.add,
                        )

                # attention per batch
                for b in range(BL):
                    bs = slice(b * TRG, (b + 1) * TRG)
                    encC = wp.tile([SRC, EMB], F32, tag="encC", name="encC", bufs=3)
                    nc.sync.dma_start(encC[:], d_encC[b])
                    pen = ps1.tile([TRG, SRC], F32, tag="en", name="pen")
                    for ke in range(ME):
                        nc.tensor.matmul(
                            pen[:], comb[ke][:, bs], encT[ke][:, bs],
                            start=(ke == 0), stop=(ke == ME - 1),
                        )
                    mx = sp.tile([TRG, 1], F32, tag="mx", name="mx")
                    nc.vector.reduce_max(mx[:], pen[:], AX.X)
                    ngb = sp.tile([TRG, 1], F32, tag="ngb", name="ngb")
                    nc.scalar.mul(ngb[:], mx[:], -SCALE)
                    ex = sp.tile([TRG, SRC], F32, tag="ex", name="ex")
                    sm = sp.tile([TRG, 1], F32, tag="sm", name="sm")
                    nc.scalar.activation(
                        ex[:], pen[:], AF.Exp, bias=ngb[:], scale=SCALE,
                        accum_out=sm[:],
                    )
                    rc = sp.tile([TRG, 1], F32, tag="rc", name="rc")
                    nc.vector.reciprocal(rc[:], sm[:])
                    att = sp.tile([TRG, SRC], F32, tag="att", name="att")
                    nc.vector.tensor_scalar_mul(att[:], ex[:], rc[:])
                    if l == n_layers - 1:
                        nc.sync.dma_start(d_att[b], att[:])
                    ptT = ps1.tile([SRC, TRG], F32, tag="tT", name="ptT")
                    nc.tensor.transpose(ptT[:], att[:], ident[:TRG, :TRG])
                    atT = sp.tile([SRC, TRG], F32, tag="atT", name="atT")
                    nc.scalar.copy(atT[:], ptT[:])
                    for me in range(ME):
                        pat = ps2.tile([128, WFC], F32, tag="ce", name="pat")
                        nc.tensor.matmul(
                            pat[:, :TRG], encC[:, me * 128:(me + 1) * 128],
                            atT[:], start=True, stop=True,
                        )
                        nc.scalar.copy(attcm[me][:, bs], pat[:, :TRG])

                # ae2h + residual epilogue:
                # x' = x*S + glu*S^2 + (ae2h(attended) + ae2h_b)*S^2
                for m in range(MH):
                    for n in range(NB):
                        cs = slice(n * NC_, (n + 1) * NC_)
                        pah = ps2.tile([128, WFC], F32, tag="ce", name="pah")
                        for ke in range(ME):
                            nc.tensor.matmul(
                                pah[:, :NC_],
                                ae2h_w[ke][:, m * 128:(m + 1) * 128],
                                attcm[ke][:, cs],
                                start=(ke == 0), stop=(ke == ME - 1),
                            )
                        s1 = tp.tile([128, NC_], F32, tag="s1", name="s1")
                        nc.scalar.activation(
                            s1[:], pah[:, :NC_], AF.Identity,
                            bias=ae2h_bs[:, m:m + 1], scale=S2,
                        )
                        nc.vector.scalar_tensor_tensor(
                            s1[:], glu[m][:, cs], S2, s1[:], OP.mult, OP.add
                        )
                        nc.vector.scalar_tensor_tensor(
                            xslice(m, n), xslice(m, n), SCALE,
                            s1[:].rearrange("p (b t) -> p b t", t=TRG),
                            OP.mult, OP.add,
                        )

            # ---- h2e (float32r), contraction split so only 4 weight tiles
            # are live at a time (2 per streaming tag) ----
            for half in range(2):
                h2e_w = []
                for j in range(4):
                    kk = half * 4 + j
                    t = cw.tile([128, CWW], F32, tag=("wa" if j < 2 else "wg"),
                                name=f"h2ew{kk}", bufs=3)
                    nc.sync.dma_start(t[:, :EMB], d_h2e[kk * 128:(kk + 1) * 128, :])
                    h2e_w.append(t)
                for me in range(ME):
                    for n in range(NB):
                        cs = slice(n * NC_, (n + 1) * NC_)
                        ph = ps2.tile([128, WFC], F32, tag="ce", name="ph")
                        for j in range(4):
                            kk = half * 4 + j
                            nc.tensor.matmul(
                                ph[:, :NC_],
                                h2e_w[j][:, me * 128:(me + 1) * 128],
                                xslice(kk, n),
                                start=(j == 0), stop=(j == 3),
                            )
                        if half == 0:
                            nc.scalar.activation(
                                cef[me][:, cs], ph[:, :NC_], AF.Identity,
                                bias=h2e_b[:, me:me + 1],
                            )
                        else:
                            nc.vector.scalar_tensor_tensor(
                                cef[me][:, cs], ph[:, :NC_], 0.0,
                                cef[me][:, cs], OP.bypass, OP.add,
                            )

            # ---- fc_out (float32r) ----
            for n in range(NFC):
                ns = slice(n * WFC, (n + 1) * WFC)
                fcb_t = fw.tile([1, WFC], BF, tag="fcb", name="fcb_t", bufs=2)
                nc.sync.dma_start(fcb_t[:], d_fcb[:, ns])
                fws = []
                for ke in range(ME):
                    t = fw.tile([128, WFC], BF, tag="fcw", name=f"fcw{ke}")
                    nc.sync.dma_start(t[:], d_fcW[ke * 128:(ke + 1) * 128, ns])
                    fws.append(t)
                for mb in range(MBT):
                    mw = min(128, BT - mb * 128)
                    pf = ps2.tile([128, WFC], F32, tag="ce", name="pf")
                    for ke in range(ME):
                        nc.tensor.matmul(
                            pf[:mw, :],
                            cef[ke][:, mb * 128:mb * 128 + mw],
                            fws[ke][:],
                            start=(ke == 0), stop=False,
                        )
                    nc.tensor.matmul(
                        pf[:mw, :], ones_1[:, :mw],
                        fcb_t[:],
                        start=False, stop=True,
                    )
                    fo = fw.tile([128, WFC], F32, tag="fo", name="fo", bufs=2)
                    nc.vector.tensor_copy(fo[:mw, :], pf[:mw, :])
                    nc.sync.dma_start(d_out[mb * 128:mb * 128 + mw, ns], fo[:mw, :])

    nc.compile()
    return nc


_CACHED = {}


def _get_nc(n_layers=NL):
    if n_layers not in _CACHED:
        _CACHED[n_layers] = build(n_layers)
    return _CACHED[n_layers]


def _prep_weights(i):
    """Host-side weight preprocessing shared by all cores."""
    f32 = np.float32
    asf = lambda x: np.asarray(x, f32)
    w = {}
    w["posT"] = np.ascontiguousarray(asf(i["pos_emb"]).T)
    w["tgtWT"] = np.ascontiguousarray(asf(i["tgt_W"]).T)
    w["tgtb"] = asf(i["tgt_b"]).reshape(TRG, 1)
    w["tokWT"] = np.ascontiguousarray(asf(i["tok_W"]).T)
    w["tokb"] = np.ascontiguousarray(asf(i["tok_b"]).reshape(ME, 128).T)
    w["e2hWT"] = np.ascontiguousarray(asf(i["e2h_W"]).T)
    w["e2hb"] = np.ascontiguousarray(asf(i["e2h_b"]).reshape(MH, 128).T)
    w["ah2eWT"] = np.ascontiguousarray(asf(i["ah2e_W"]).T)
    w["ah2eb"] = np.ascontiguousarray(asf(i["ah2e_b"]).reshape(ME, 128).T)
    w["ae2hWT"] = np.ascontiguousarray(asf(i["ae2h_W"]).T)
    w["ae2hbs"] = np.ascontiguousarray(
        (asf(i["ae2h_b"]) * np.float32(S2)).reshape(MH, 128).T
    )
    w["h2eWT"] = np.ascontiguousarray(asf(i["h2e_W"]).T)
    w["h2eb"] = np.ascontiguousarray(asf(i["h2e_b"]).reshape(ME, 128).T)
    w["fcWT"] = np.ascontiguousarray(asf(i["fc_W"]).T).astype(BF16)
    w["fcb"] = asf(i["fc_b"]).reshape(1, OUT).astype(BF16)
    cW = asf(i["conv_W"])  # [NL, 2H, H, K]
    g0, g1, g2 = cW[..., 0], cW[..., 1], cW[..., 2]
    cw4 = np.stack(
        [g0, (g0 + g1 + g2) * np.float32(0.5),
         (g0 - g1 + g2) * np.float32(0.5), g2], axis=-1)  # [NL, 2H, H, 4]
    # [l, mt, q, kk, p, i] -> [l, mt, p, (i, kk, q)]
    w["convWT"] = np.ascontiguousarray(
        cw4.reshape(NL, 2 * MH, 128, MH, 128, NWI).transpose(0, 1, 4, 5, 3, 2)
    ).reshape(NL, 2 * MH, 128, NWI * MH * 128)
    cb_ = asf(i["conv_b"])  # [NL, 2H]
    w["cba"] = np.ascontiguousarray(cb_[:, :HID].reshape(NL, MH, 128).transpose(0, 2, 1))
    w["cbg"] = np.ascontiguousarray(cb_[:, HID:].reshape(NL, MH, 128).transpose(0, 2, 1))
    return w


LAST_EXEC_NS = None


def _maybe_enable_trace():
    """Register the NTFF profile hook (missing antenv.axon_hooks shim)."""
    try:
        import antenv.axon_hooks  # noqa: F401
        return True
    except ImportError:
        pass
    try:
        import types
        import antenv
        from trn_agent_boot.trn_boot import _ntff_profile_via_ctypes

        hook = _ntff_profile_via_ctypes("/opt/axon/libaxon_pjrt.so")
        mod = types.ModuleType("antenv.axon_hooks")
        _state = {"hook": hook}
        mod.set_axon_ntff_profile_hook = lambda h: _state.__setitem__("hook", h)
        mod.get_axon_ntff_profile_hook = lambda: _state["hook"]
        sys.modules["antenv.axon_hooks"] = mod
        antenv.axon_hooks = mod
        return hook is not None
    except Exception:
        return False


def kernel(**inputs):
    global LAST_EXEC_NS
    n_layers = int(os.environ.get("KERNEL_NL", NL))
    trace = os.environ.get("KERNEL_TRACE", "0") == "1"
    if trace:
        trace = _maybe_enable_trace()

    nc = _get_nc(n_layers)
    w = _prep_weights(inputs)

    enc_conved = np.asarray(inputs["encoder_conved"], np.float32)
    enc_combined = np.asarray(inputs["encoder_combined"], np.float32)

    in_maps = []
    for c in range(CORES):
        sh = slice(c * BL, (c + 1) * BL)
        m = dict(w)
        m["encT"] = np.ascontiguousarray(
            enc_conved[sh].transpose(2, 0, 1).reshape(EMB, BT)
        )
        m["encC"] = np.ascontiguousarray(enc_combined[sh])
        in_maps.append(m)

    res = run_bass_kernel_spmd(
        nc, in_maps, list(range(CORES)), trace=trace,
        trace_cores=[0] if trace else None,
    )
    LAST_EXEC_NS = res.exec_time_ns

    out = np.empty((B, TRG, OUT), np.float32)
    att = np.empty((B, TRG, SRC), np.float32)
    for c in range(CORES):
        out[c * BL:(c + 1) * BL] = res.results[c]["out"].reshape(BL, TRG, OUT)
        att[c * BL:(c + 1) * BL] = res.results[c]["att_out"]
    return out, att


# revision 16
# speedup vs baseline: 1.7085x; 1.2891x over previous
"""Trainium2 Bass kernel for nn_AnswerEmbedding (conv-seq2seq decoder block).

Strategy: pure data-parallel over batch (64 -> 8 per NeuronCore), no
collectives.  All activations stay on-chip in channel-major layout
[C, B*T]; weights are pre-transposed host-side into lhsT layouts.

Precision: the model is chaotic — the attention softmax sharpens over 8
layers and amplifies per-element noise ~600x into the output, so every
in-layer matmul runs in true fp32 (4 cycles/row on the PE).  Only the
final h2e+fc projections (whose errors do not amplify) run as float32r
(TF32-class, 1 cycle/row at N>=256).
"""

import os
import sys

import numpy as np
import ml_dtypes

for _p in ("/opt/trn_rl_repo",):
    if os.path.isdir(_p) and _p not in sys.path:
        sys.path.insert(0, _p)

import concourse.bass as bass
from concourse import bacc, masks, mybir, tile
from concourse.bass_utils import run_bass_kernel_spmd

B, SRC, EMB, HID, TRG, OUT, NL, K = 64, 100, 512, 1024, 100, 10000, 8, 3
CORES = 8
BL = B // CORES          # batches per core
BT = BL * TRG            # 800 local columns
TP = TRG + 2             # padded columns per batch
NB = 2                   # column chunks for N<=512 matmuls
NC_ = BT // NB           # 400 columns per chunk
BPC = BL // NB           # 4 batches per chunk
SCALE = float(np.float32(np.sqrt(np.float32(0.3))))
S2 = SCALE * SCALE

F32 = mybir.dt.float32
F32R = mybir.dt.float32r
BF = mybir.dt.bfloat16
BF16 = ml_dtypes.bfloat16
AF = mybir.ActivationFunctionType
OP = mybir.AluOpType
AX = mybir.AxisListType

ME = EMB // 128          # 4  E tiles
MH = HID // 128          # 8  HID tiles
NTAP = K * MH            # 24 contraction tiles per conv output tile (direct form)
NWI = 4                  # Winograd F(2,3) transforms
WJ = TRG // 2            # 50 output pairs per batch
WN = BPC * WJ            # 200 columns per Winograd matmul (per n-chunk)
NFC = 20                 # fc column chunks
WFC = OUT // NFC         # 500
MBT = (BT + 127) // 128  # 7 bt tiles for fc


def build(n_layers=NL):
    nc = bacc.Bacc(None, target_bir_lowering=False, debug=False)

    # ---- DRAM parameters (per-core shards / replicated weights) ----
    d_encT = nc.declare_dram_parameter("encT", [EMB, BT], F32, isOutput=False)
    d_encC = nc.declare_dram_parameter("encC", [BL, SRC, EMB], F32, isOutput=False)
    d_pos = nc.declare_dram_parameter("posT", [EMB, TRG], F32, isOutput=False)
    d_tgtW = nc.declare_dram_parameter("tgtWT", [SRC, TRG], F32, isOutput=False)
    d_tgtb = nc.declare_dram_parameter("tgtb", [TRG, 1], F32, isOutput=False)
    d_tokW = nc.declare_dram_parameter("tokWT", [TRG, EMB], F32, isOutput=False)
    d_tokb = nc.declare_dram_parameter("tokb", [128, ME], F32, isOutput=False)
    d_e2h = nc.declare_dram_parameter("e2hWT", [EMB, HID], F32, isOutput=False)
    d_e2hb = nc.declare_dram_parameter("e2hb", [128, MH], F32, isOutput=False)
    d_ah2eN = nc.declare_dram_parameter("ah2eWN", [EMB, HID], F32, isOutput=False)
    d_ah2eb = nc.declare_dram_parameter("ah2eb", [128, ME], F32, isOutput=False)
    d_ae2h = nc.declare_dram_parameter("ae2hWT", [EMB, HID], F32, isOutput=False)
    d_ae2hb = nc.declare_dram_parameter("ae2hbs", [128, MH], F32, isOutput=False)
    d_h2e = nc.declare_dram_parameter("h2eWT", [HID, EMB], F32, isOutput=False)
    d_h2eb = nc.declare_dram_parameter("h2eb", [128, ME], F32, isOutput=False)
    d_fcW = nc.declare_dram_parameter("fcWT", [EMB, OUT], BF, isOutput=False)
    d_fcb = nc.declare_dram_parameter("fcb", [1, OUT], BF, isOutput=False)
    # [l, out-tile(16), p(128 contraction rows), (k, kk, q) = 24*128 cols]
    d_convW = nc.declare_dram_parameter(
        "convWT", [NL, 2 * MH, 128, NWI * MH * 128], F32, isOutput=False
    )
    d_cba = nc.declare_dram_parameter("cba", [NL, 128, MH], F32, isOutput=False)
    d_cbg = nc.declare_dram_parameter("cbg", [NL, 128, MH], F32, isOutput=False)

    d_out = nc.declare_dram_parameter("out", [BT, OUT], F32, isOutput=True)
    d_att = nc.declare_dram_parameter("att_out", [BL, TRG, SRC], F32, isOutput=True)

    with tile.TileContext(nc) as tc:
        with (
            tc.tile_pool(name="wp", bufs=1) as wp,
            tc.tile_pool(name="st", bufs=1) as st,
            tc.tile_pool(name="cw", bufs=3) as cw,
            tc.tile_pool(name="cb", bufs=2) as cb,
            tc.tile_pool(name="fw", bufs=5) as fw,
            tc.tile_pool(name="tp", bufs=2) as tp,
            tc.tile_pool(name="sp", bufs=2) as sp,
            tc.tile_pool(name="ps2", bufs=2, space=bass.MemorySpace.PSUM) as ps2,
            tc.tile_pool(name="ps1", bufs=1, space=bass.MemorySpace.PSUM) as ps1,
        ):
            CWW = MH * 128  # 1024 cols per conv weight chunk (one transform)

            # ---- static weights into SBUF ----
            def loadw(dram, parts, width, tagn):
                tiles = []
                for i in range(parts):
                    t = wp.tile([128, width], F32, tag=f"{tagn}{i}", name=f"{tagn}{i}")
                    nc.sync.dma_start(t[:], dram[i * 128:(i + 1) * 128, :])
                    tiles.append(t)
                return tiles

            ae2h_w = loadw(d_ae2h, ME, HID, "ae2h")
            pos_w = loadw(d_pos, ME, TRG, "pos")
            encT = loadw(d_encT, ME, BT, "encT")

            tgt_w = wp.tile([SRC, TRG], F32, tag="tgtw", name="tgt_w")
            nc.sync.dma_start(tgt_w[:], d_tgtW[:])
            tok_w = wp.tile([TRG, EMB], F32, tag="tokw", name="tok_w")
            nc.sync.dma_start(tok_w[:], d_tokW[:])
            tgt_b = wp.tile([TRG, 1], F32, tag="tgtb", name="tgt_b")
            nc.sync.dma_start(tgt_b[:], d_tgtb[:])
            tok_b = wp.tile([128, ME], F32, tag="tokb", name="tok_b")
            nc.sync.dma_start(tok_b[:], d_tokb[:])
            e2h_b = wp.tile([128, MH], F32, tag="e2hb", name="e2h_b")
            nc.sync.dma_start(e2h_b[:], d_e2hb[:])
            ah2e_b = wp.tile([128, ME], F32, tag="ah2eb", name="ah2e_b")
            nc.sync.dma_start(ah2e_b[:], d_ah2eb[:])
            ae2h_bs = wp.tile([128, MH], F32, tag="ae2hbs", name="ae2h_bs")
            nc.sync.dma_start(ae2h_bs[:], d_ae2hb[:])
            h2e_b = wp.tile([128, ME], F32, tag="h2eb", name="h2e_b")
            nc.sync.dma_start(h2e_b[:], d_h2eb[:])

            ident = wp.tile([128, 128], F32, tag="ident", name="ident")
            masks.make_identity(nc, ident[:])
            ones_e = wp.tile([128, 1], F32, tag="ones_e", name="ones_e")
            nc.gpsimd.memset(ones_e[:], 1.0 / EMB)
            ones_1 = wp.tile([1, 128], BF, tag="ones_1", name="ones_1")
            nc.gpsimd.memset(ones_1[:], 1.0)

            # ---- persistent activation state ----
            # xf: fp32 residual stream in zero-padded layout [128, 8*(100+2)]
            xf = [st.tile([128, BL * TP], F32, tag=f"xf{i}", name=f"xf{i}") for i in range(MH)]
            glu = [st.tile([128, BT], F32, tag=f"glu{i}", name=f"glu{i}") for i in range(MH)]
            emb = [st.tile([128, BT], F32, tag=f"emb{i}", name=f"emb{i}") for i in range(ME)]
            Q = [st.tile([128, BT], F32, tag=f"q{i}", name=f"q{i}") for i in range(MH)]
            cst = [st.tile([TRG, SRC], F32, tag=f"cst{i}", name=f"cst{i}") for i in range(BL)]
            attcm = [st.tile([128, BT], F32, tag=f"attcm{i}", name=f"attcm{i}") for i in range(ME)]
            tok_sb = [st.tile([128, BL], F32, tag=f"tok{i}", name=f"tok{i}") for i in range(ME)]
            cef = [st.tile([128, BT], BF, tag=f"cef{i}", name=f"cef{i}") for i in range(ME)]

            for i in range(MH):
                nc.gpsimd.memset(xf[i][:], 0.0)

            def xf3(i):
                return xf[i][:].rearrange("p (b t) -> p b t", t=TP)

            def xslice(i, n, k=1, w=TRG):
                # data columns of chunk n with tap offset k (k=1 => aligned)
                return xf3(i)[:, n * BPC:(n + 1) * BPC, k:k + w]

            # ---- prologue: ec_mean -> hidden_target -> tok -> embedded ----
            psec = ps1.tile([SRC, SRC], F32, tag="en", name="psec")
            for b in range(BL):
                for ke in range(ME):
                    nc.tensor.matmul(
                        psec[:, b:b + 1],
                        encT[ke][:, b * TRG:(b + 1) * TRG],
                        ones_e[:],
                        start=(ke == 0), stop=(ke == ME - 1),
                    )
            ec_sb = sp.tile([SRC, BL], F32, tag="ec", name="ec_sb")
            nc.scalar.copy(ec_sb[:], psec[:, :BL])

            psht = ps1.tile([SRC, SRC], F32, tag="en", name="psht")
            nc.tensor.matmul(psht[:TRG, :BL], tgt_w[:], ec_sb[:], start=True, stop=True)
            ht_sb = sp.tile([TRG, BL], F32, tag="ht", name="ht_sb")
            nc.scalar.activation(ht_sb[:], psht[:TRG, :BL], AF.Identity, bias=tgt_b[:])

            for me in range(ME):
                pstok = ps2.tile([128, WFC], F32, tag="ce", name="pstok")
                nc.tensor.matmul(
                    pstok[:, :BL], tok_w[:, me * 128:(me + 1) * 128], ht_sb[:],
                    start=True, stop=True,
                )
                nc.scalar.activation(
                    tok_sb[me][:], pstok[:, :BL], AF.Identity,
                    bias=tok_b[:, me:me + 1],
                )

            for me in range(ME):
                for b in range(BL):
                    nc.scalar.activation(
                        emb[me][:, b * TRG:(b + 1) * TRG], pos_w[me][:],
                        AF.Identity, bias=tok_sb[me][:, b:b + 1],
                    )

            # conv_input = e2h(embedded); e2h weights stream through cw slots
            e2h_w = []
            for ke in range(ME):
                t = cw.tile([128, CWW], F32, tag=("wa" if ke < 2 else "wg"),
                            name=f"e2hw{ke}", bufs=3)
                nc.sync.dma_start(t[:], d_e2h[ke * 128:(ke + 1) * 128, :])
                e2h_w.append(t)
            for kk in range(MH):
                for n in range(NB):
                    pse = ps2.tile([128, WFC], F32, tag="ce", name="pse")
                    for ke in range(ME):
                        nc.tensor.matmul(
                            pse[:, :NC_],
                            e2h_w[ke][:, kk * 128:(kk + 1) * 128],
                            emb[ke][:, n * NC_:(n + 1) * NC_],
                            start=(ke == 0), stop=(ke == ME - 1),
                        )
                    nc.scalar.activation(
                        xslice(kk, n),
                        pse[:, :NC_].rearrange("p (b t) -> p b t", t=TRG),
                        AF.Identity, bias=e2h_b[:, kk:kk + 1],
                    )

            # ---- fold ah2e into energy: Q = ah2e_W^T-projected encoder,
            # const_b = (embedded + ah2e_b)^T . encT  (both layer-independent) ----
            ah2eN_w = []
            for ke in range(ME):
                t = cw.tile([128, CWW], F32, tag=("wa" if ke < 2 else "wg"),
                            name=f"ah2eN{ke}", bufs=3)
                nc.sync.dma_start(t[:], d_ah2eN[ke * 128:(ke + 1) * 128, :])
                ah2eN_w.append(t)
            for kk in range(MH):
                for n in range(NB):
                    cs = slice(n * NC_, (n + 1) * NC_)
                    pq = ps2.tile([128, WFC], F32, tag="ce", name="pq")
                    for ke in range(ME):
                        nc.tensor.matmul(
                            pq[:, :NC_],
                            ah2eN_w[ke][:, kk * 128:(kk + 1) * 128],
                            encT[ke][:, cs],
                            start=(ke == 0), stop=(ke == ME - 1),
                        )
                    nc.vector.tensor_copy(Q[kk][:, cs], pq[:, :NC_])
            for me in range(ME):
                nc.scalar.activation(
                    glu[me][:], emb[me][:], AF.Identity,
                    bias=ah2e_b[:, me:me + 1],
                )
            for b in range(BL):
                bs = slice(b * TRG, (b + 1) * TRG)
                pcst = ps1.tile([TRG, SRC], F32, tag="en", name="pcst")
                for ke in range(ME):
                    nc.tensor.matmul(
                        pcst[:], glu[ke][:, bs], encT[ke][:, bs],
                        start=(ke == 0), stop=(ke == ME - 1),
                    )
                nc.scalar.copy(cst[b][:], pcst[:])

            # ---- layers ----
            for l in range(n_layers):
                cba = cb.tile([128, MH], F32, tag="cba", name="cba")
                nc.sync.dma_start(cba[:], d_cba[l])
                cbg = cb.tile([128, MH], F32, tag="cbg", name="cbg")
                nc.sync.dma_start(cbg[:], d_cbg[l])

                # conv + GLU via Winograd F(2,3): y[2j]=m1+m2+m3,
                # y[2j+1]=m2-m3-m4, mi = Gi(W) . Bi(x) over channels.
                scr = emb + attcm  # scratch for transformed inputs (emb is prologue-only now)
                for n in range(NB):
                    br = slice(n * BPC, (n + 1) * BPC)
                    for kk in range(MH):
                        xf4 = xf[kk][:].rearrange("p (b u v) -> p b u v", u=TP // 2, v=2)
                        d0 = xf4[:, br, 0:WJ, 0]
                        d1 = xf4[:, br, 0:WJ, 1]
                        d2 = xf4[:, br, 1:WJ + 1, 0]
                        d3 = xf4[:, br, 1:WJ + 1, 1]
                        for i, (pa, pb, op) in enumerate(
                            [(d0, d2, OP.subtract), (d1, d2, OP.add),
                             (d2, d1, OP.subtract), (d1, d3, OP.subtract)]
                        ):
                            o3 = scr[kk][:, i * WN:(i + 1) * WN].rearrange(
                                "p (b j) -> p b j", j=WJ)
                            nc.vector.scalar_tensor_tensor(
                                o3, pa, 0.0, pb, OP.bypass, op)
                    for m in range(MH):
                        gl4 = glu[m][:].rearrange("p (b u v) -> p b u v", u=WJ, v=2)
                        # a-half -> SBUF temps, g-half stays in PSUM
                        psA = []
                        for i in range(NWI):
                            wa = cw.tile([128, CWW], F32, tag="wa", name="wa", bufs=3)
                            nc.sync.dma_start(
                                wa[:], d_convW[l, m, :, i * CWW:(i + 1) * CWW])
                            ps = ps2.tile([128, WN], F32, tag=f"m{i}",
                                          name=f"psA{i}", bufs=1)
                            for kk in range(MH):
                                nc.tensor.matmul(
                                    ps[:], wa[:, kk * 128:(kk + 1) * 128],
                                    scr[kk][:, i * WN:(i + 1) * WN],
                                    start=(kk == 0), stop=(kk == MH - 1),
                                )
                            psA.append(ps)
                        c1 = tp.tile([128, WN], F32, tag="cc", name="c1")
                        nc.vector.tensor_copy(c1[:], psA[1][:])
                        tva = tp.tile([128, WN], F32, tag="tva", name="tva")
                        nc.vector.scalar_tensor_tensor(
                            tva[:], psA[0][:], 0.0, c1[:], OP.bypass, OP.add)
                        nc.vector.scalar_tensor_tensor(
                            tva[:], psA[2][:], 0.0, tva[:], OP.bypass, OP.add)
                        tvb = tp.tile([128, WN], F32, tag="tvb", name="tvb")
                        nc.vector.scalar_tensor_tensor(
                            tvb[:], psA[2][:], -1.0, c1[:], OP.mult, OP.add)
                        nc.vector.scalar_tensor_tensor(
                            tvb[:], psA[3][:], -1.0, tvb[:], OP.mult, OP.add)
                        psG = []
                        for i in range(NWI):
                            wg = cw.tile([128, CWW], F32, tag="wg", name="wg", bufs=3)
                            nc.sync.dma_start(
                                wg[:], d_convW[l, MH + m, :, i * CWW:(i + 1) * CWW])
                            ps = ps2.tile([128, WN], F32, tag=f"m{i}",
                                          name=f"psG{i}", bufs=1)
                            for kk in range(MH):
                                nc.tensor.matmul(
                                    ps[:], wg[:, kk * 128:(kk + 1) * 128],
                                    scr[kk][:, i * WN:(i + 1) * WN],
                                    start=(kk == 0), stop=(kk == MH - 1),
                                )
                            psG.append(ps)
                        c2 = tp.tile([128, WN], F32, tag="cc", name="c2")
                        nc.vector.tensor_copy(c2[:], psG[1][:])
                        tvc = tp.tile([128, WN], F32, tag="tvc", name="tvc")
                        nc.vector.scalar_tensor_tensor(
                            tvc[:], psG[0][:], 0.0, c2[:], OP.bypass, OP.add)
                        nc.vector.scalar_tensor_tensor(
                            tvc[:], psG[2][:], 0.0, tvc[:], OP.bypass, OP.add)
                        tvd = tp.tile([128, WN], F32, tag="tvd", name="tvd")
                        nc.vector.scalar_tensor_tensor(
                            tvd[:], psG[2][:], -1.0, c2[:], OP.mult, OP.add)
                        nc.vector.scalar_tensor_tensor(
                            tvd[:], psG[3][:], -1.0, tvd[:], OP.mult, OP.add)
                        nc.scalar.activation(
                            tvc[:], tvc[:], AF.Sigmoid, bias=cbg[:, m:m + 1])
                        nc.vector.scalar_tensor_tensor(
                            gl4[:, br, :, 0], tva[:], cba[:, m:m + 1], tvc[:],
                            OP.add, OP.mult)
                        nc.scalar.activation(
                            tvd[:], tvd[:], AF.Sigmoid, bias=cbg[:, m:m + 1])
                        nc.vector.scalar_tensor_tensor(
                            gl4[:, br, :, 1], tvb[:], cba[:, m:m + 1], tvd[:],
                            OP.add, OP.mult)

                # attention per batch
                for b in range(BL):
                    bs = slice(b * TRG, (b + 1) * TRG)
                    encC = wp.tile([SRC, EMB], F32, tag="encC", name="encC", bufs=3)
                    nc.sync.dma_start(encC[:], d_encC[b])
                    pen = ps1.tile([TRG, SRC], F32, tag="en", name="pen")
                    for kk in range(MH):
                        nc.tensor.matmul(
                            pen[:], glu[kk][:, bs], Q[kk][:, bs],
                            start=(kk == 0), stop=(kk == MH - 1),
                        )
                    es = sp.tile([TRG, SRC], F32, tag="es", name="es")
                    nc.vector.scalar_tensor_tensor(
                        es[:], pen[:], 0.0, cst[b][:], OP.bypass, OP.add)
                    mx = sp.tile([TRG, 1], F32, tag="mx", name="mx")
                    nc.vector.reduce_max(mx[:], es[:], AX.X)
                    ngb = sp.tile([TRG, 1], F32, tag="ngb", name="ngb")
                    nc.scalar.mul(ngb[:], mx[:], -SCALE)
                    ex = sp.tile([TRG, SRC], F32, tag="ex", name="ex")
                    sm = sp.tile([TRG, 1], F32, tag="sm", name="sm")
                    nc.scalar.activation(
                        ex[:], es[:], AF.Exp, bias=ngb[:], scale=SCALE,
                        accum_out=sm[:],
                    )
                    rc = sp.tile([TRG, 1], F32, tag="rc", name="rc")
                    nc.vector.reciprocal(rc[:], sm[:])
                    att = sp.tile([TRG, SRC], F32, tag="att", name="att")
                    nc.vector.tensor_scalar_mul(att[:], ex[:], rc[:])
                    if l == n_layers - 1:
                        nc.sync.dma_start(d_att[b], att[:])
                    ptT = ps1.tile([SRC, TRG], F32, tag="tT", name="ptT")
                    nc.tensor.transpose(ptT[:], att[:], ident[:TRG, :TRG])
                    atT = sp.tile([SRC, TRG], F32, tag="atT", name="atT")
                    nc.scalar.copy(atT[:], ptT[:])
                    for me in range(ME):
                        pat = ps2.tile([128, WFC], F32, tag="ce", name="pat")
                        nc.tensor.matmul(
                            pat[:, :TRG], encC[:, me * 128:(me + 1) * 128],
                            atT[:], start=True, stop=True,
                        )
                        nc.scalar.copy(attcm[me][:, bs], pat[:, :TRG])

                # ae2h + residual epilogue:
                # x' = x*S + glu*S^2 + (ae2h(attended) + ae2h_b)*S^2
                for m in range(MH):
                    for n in range(NB):
                        cs = slice(n * NC_, (n + 1) * NC_)
                        pah = ps2.tile([128, WFC], F32, tag="ce", name="pah")
                        for ke in range(ME):
                            nc.tensor.matmul(
                                pah[:, :NC_],
                                ae2h_w[ke][:, m * 128:(m + 1) * 128],
                                attcm[ke][:, cs],
                                start=(ke == 0), stop=(ke == ME - 1),
                            )
                        s1 = tp.tile([128, NC_], F32, tag="s1", name="s1")
                        nc.scalar.activation(
                            s1[:], pah[:, :NC_], AF.Identity,
                            bias=ae2h_bs[:, m:m + 1], scale=S2,
                        )
                        nc.vector.scalar_tensor_tensor(
                            s1[:], glu[m][:, cs], S2, s1[:], OP.mult, OP.add
                        )
                        nc.vector.scalar_tensor_tensor(
                            xslice(m, n), xslice(m, n), SCALE,
                            s1[:].rearrange("p (b t) -> p b t", t=TRG),
                            OP.mult, OP.add,
                        )

            # ---- h2e (float32r), contraction split so only 4 weight tiles
            # are live at a time (2 per streaming tag) ----
            for half in range(2):
                h2e_w = []
                for j in range(4):
                    kk = half * 4 + j
                    t = cw.tile([128, CWW], F32, tag=("wa" if j < 2 else "wg"),
                                name=f"h2ew{kk}", bufs=3)
                    nc.sync.dma_start(t[:, :EMB], d_h2e[kk * 128:(kk + 1) * 128, :])
                    h2e_w.append(t)
                for me in range(ME):
                    for n in range(NB):
                        cs = slice(n * NC_, (n + 1) * NC_)
                        ph = ps2.tile([128, WFC], F32, tag="ce", name="ph")
                        for j in range(4):
                            kk = half * 4 + j
                            nc.tensor.matmul(
                                ph[:, :NC_],
                                h2e_w[j][:, me * 128:(me + 1) * 128],
                                xslice(kk, n),
                                start=(j == 0), stop=(j == 3),
                            )
                        if half == 0:
                            nc.scalar.activation(
                                cef[me][:, cs], ph[:, :NC_], AF.Identity,
                                bias=h2e_b[:, me:me + 1],
                            )
                        else:
                            nc.vector.scalar_tensor_tensor(
                                cef[me][:, cs], ph[:, :NC_], 0.0,
                                cef[me][:, cs], OP.bypass, OP.add,
                            )

            # ---- fc_out (float32r) ----
            for n in range(NFC):
                ns = slice(n * WFC, (n + 1) * WFC)
                fcb_t = fw.tile([1, WFC], BF, tag="fcb", name="fcb_t", bufs=2)
                nc.sync.dma_start(fcb_t[:], d_fcb[:, ns])
                fws = []
                for ke in range(ME):
                    t = fw.tile([128, WFC], BF, tag="fcw", name=f"fcw{ke}")
                    nc.sync.dma_start(t[:], d_fcW[ke * 128:(ke + 1) * 128, ns])
                    fws.append(t)
                for mb in range(MBT):
                    mw = min(128, BT - mb * 128)
                    pf = ps2.tile([128, WFC], F32, tag="ce", name="pf")
                    for ke in range(ME):
                        nc.tensor.matmul(
                            pf[:mw, :],
                            cef[ke][:, mb * 128:mb * 128 + mw],
                            fws[ke][:],
                            start=(ke == 0), stop=False,
                        )
                    nc.tensor.matmul(
                        pf[:mw, :], ones_1[:, :mw],
                        fcb_t[:],
                        start=False, stop=True,
                    )
                    fo = fw.tile([128, WFC], F32, tag="fo", name="fo", bufs=2)
                    nc.vector.tensor_copy(fo[:mw, :], pf[:mw, :])
                    nc.sync.dma_start(d_out[mb * 128:mb * 128 + mw, ns], fo[:mw, :])

    nc.compile()
    return nc


_CACHED = {}


def _get_nc(n_layers=NL):
    if n_layers not in _CACHED:
        _CACHED[n_layers] = build(n_layers)
    return _CACHED[n_layers]


def _prep_weights(i):
    """Host-side weight preprocessing shared by all cores."""
    f32 = np.float32
    asf = lambda x: np.asarray(x, f32)
    w = {}
    w["posT"] = np.ascontiguousarray(asf(i["pos_emb"]).T)
    w["tgtWT"] = np.ascontiguousarray(asf(i["tgt_W"]).T)
    w["tgtb"] = asf(i["tgt_b"]).reshape(TRG, 1)
    w["tokWT"] = np.ascontiguousarray(asf(i["tok_W"]).T)
    w["tokb"] = np.ascontiguousarray(asf(i["tok_b"]).reshape(ME, 128).T)
    w["e2hWT"] = np.ascontiguousarray(asf(i["e2h_W"]).T)
    w["e2hb"] = np.ascontiguousarray(asf(i["e2h_b"]).reshape(MH, 128).T)
    w["ah2eWN"] = np.ascontiguousarray(asf(i["ah2e_W"]))
    w["ah2eb"] = np.ascontiguousarray(asf(i["ah2e_b"]).reshape(ME, 128).T)
    w["ae2hWT"] = np.ascontiguousarray(asf(i["ae2h_W"]).T)
    w["ae2hbs"] = np.ascontiguousarray(
        (asf(i["ae2h_b"]) * np.float32(S2)).reshape(MH, 128).T
    )
    w["h2eWT"] = np.ascontiguousarray(asf(i["h2e_W"]).T)
    w["h2eb"] = np.ascontiguousarray(asf(i["h2e_b"]).reshape(ME, 128).T)
    w["fcWT"] = np.ascontiguousarray(asf(i["fc_W"]).T).astype(BF16)
    w["fcb"] = asf(i["fc_b"]).reshape(1, OUT).astype(BF16)
    cW = asf(i["conv_W"])  # [NL, 2H, H, K]
    g0, g1, g2 = cW[..., 0], cW[..., 1], cW[..., 2]
    cw4 = np.stack(
        [g0, (g0 + g1 + g2) * np.float32(0.5),
         (g0 - g1 + g2) * np.float32(0.5), g2], axis=-1)  # [NL, 2H, H, 4]
    # [l, mt, q, kk, p, i] -> [l, mt, p, (i, kk, q)]
    w["convWT"] = np.ascontiguousarray(
        cw4.reshape(NL, 2 * MH, 128, MH, 128, NWI).transpose(0, 1, 4, 5, 3, 2)
    ).reshape(NL, 2 * MH, 128, NWI * MH * 128)
    cb_ = asf(i["conv_b"])  # [NL, 2H]
    w["cba"] = np.ascontiguousarray(cb_[:, :HID].reshape(NL, MH, 128).transpose(0, 2, 1))
    w["cbg"] = np.ascontiguousarray(cb_[:, HID:].reshape(NL, MH, 128).transpose(0, 2, 1))
    return w


LAST_EXEC_NS = None


def _maybe_enable_trace():
    """Register the NTFF profile hook (missing antenv.axon_hooks shim)."""
    try:
        import antenv.axon_hooks  # noqa: F401
        return True
    except ImportError:
        pass
    try:
        import types
        import antenv
        from trn_agent_boot.trn_boot import _ntff_profile_via_ctypes

        hook = _ntff_profile_via_ctypes("/opt/axon/libaxon_pjrt.so")
        mod = types.ModuleType("antenv.axon_hooks")
        _state = {"hook": hook}
        mod.set_axon_ntff_profile_hook = lambda h: _state.__setitem__("hook", h)
        mod.get_axon_ntff_profile_hook = lambda: _state["hook"]
        sys.modules["antenv.axon_hooks"] = mod
        antenv.axon_hooks = mod
        return hook is not None
    except Exception:
        return False


def kernel(**inputs):
    global LAST_EXEC_NS
    n_layers = int(os.environ.get("KERNEL_NL", NL))
    trace = os.environ.get("KERNEL_TRACE", "0") == "1"
    if trace:
        trace = _maybe_enable_trace()

    nc = _get_nc(n_layers)
    w = _prep_weights(inputs)

    enc_conved = np.asarray(inputs["encoder_conved"], np.float32)
    enc_combined = np.asarray(inputs["encoder_combined"], np.float32)

    in_maps = []
    for c in range(CORES):
        sh = slice(c * BL, (c + 1) * BL)
        m = dict(w)
        m["encT"] = np.ascontiguousarray(
            enc_conved[sh].transpose(2, 0, 1).reshape(EMB, BT)
        )
        m["encC"] = np.ascontiguousarray(enc_combined[sh])
        in_maps.append(m)

    res = run_bass_kernel_spmd(
        nc, in_maps, list(range(CORES)), trace=trace,
        trace_cores=[0] if trace else None,
    )
    LAST_EXEC_NS = res.exec_time_ns

    out = np.empty((B, TRG, OUT), np.float32)
    att = np.empty((B, TRG, SRC), np.float32)
    for c in range(CORES):
        out[c * BL:(c + 1) * BL] = res.results[c]["out"].reshape(BL, TRG, OUT)
        att[c * BL:(c + 1) * BL] = res.results[c]["att_out"]
    return out, att
